# revision 13
# baseline (speedup 1.0000x reference)
"""Trainium2 Bass kernel for a GPT-style transformer block.

Reference computation (per batch element):
    h  = LN1(x);  qkv = h @ qkv_w + qkv_b
    att = causal_softmax(q @ k.T / sqrt(64));  o = att @ v
    x  = x + o @ out_w + out_b
    h  = LN2(x);  u = relu(h @ fc_w + fc_b)
    y  = x + u @ proj_w + proj_b

Shapes: x [16, 1024, 256], 4 heads x 64, MLP hidden 1024.

Strategy: pure data-parallel over batch, 2 batch elements per core on 8
cores, no collectives.  Within a core:
  - LN gamma/beta folded into the following matmul weights on the host.
  - The residual stream (xs/x2) and LN math stay fp32; matmul operands
    (h1T/qkT/vv/expT/uT + weights) are bf16 except out_proj (f32r).
  - Attention scores are computed transposed (scoresT[k, q]) so the
    causal region is per-k-tile a contiguous q-range.  The causal mask is
    ADDITIVE (-1e30) applied by DVE on the PSUM diagonal block before the
    scalar-engine exp, so P@V waits only on exps.  Softmax denominators
    come from an extra all-ones column appended to V (P@V_aug also yields
    row sums); normalization uses a sums -> reciprocal DRAM round trip
    that lands per-partition reciprocal columns broadcast over the o rows.
  - Biases ride evacuation ops (DVE adds against broadcast bias tiles,
    out_b pre-added into the residual) instead of rank-1 PE matmuls
    (except proj_b, whose residual target also feeds LN2).
  - Emission is generator-driven and interleaved so each engine's
    in-order queue rarely has a blocked op ahead of ready work:
    p1(0)+p1(1) | p2(0)+p2(1) | p2(1)+p3(0) | p3(0)+p3(1) | p3(1).
"""

import sys

sys.path.insert(0, "/opt/trn_rl_repo")

import numpy as np

import concourse.bass as bass
import concourse.bacc as bacc
import concourse.tile as tile
from concourse import mybir
from concourse.masks import make_identity, make_lower_triangular

# The activation-set chooser is first-match, which thrashes table loads
# between the exp and ln sets.  natural_log_exp_and_others contains every
# function this kernel uses (exp, ln, copy/identity, relu), so restrict
# the chooser to it (other entries stay in place so set ids keep their
# positions for walrus).
_orig_get_act_tables = bacc.get_activation_tables


def _one_set_tables(module_arch):
    tabs = _orig_get_act_tables(module_arch)
    return {name: (fns if name == "natural_log_exp_and_others" else set())
            for name, fns in tabs.items()}


bacc.get_activation_tables = _one_set_tables

F32 = mybir.dt.float32
F32R = mybir.dt.float32r
BF16 = mybir.dt.bfloat16
AF = mybir.ActivationFunctionType
ALU = mybir.AluOpType

NCORES = 8
B = 16
BPC = B // NCORES  # 2 batch elements per core
S = 1024
E = 256
H = 4
D = 64
FF = 1024
ST = S // 128  # 8 seq tiles
ET = E // 128  # 2 feature tiles
FT = FF // 128  # 8 mlp-hidden tiles
EPS = 1e-5
NEG = -1.0e30

# ragged offsets for the causal expT store: tile j holds q in [128j, S)
EOFF = [0]
for _j in range(ST):
    EOFF.append(EOFF[-1] + (S - 128 * _j))
ETOT = EOFF[ST]  # 4608


def _r(ap):
    """View an fp32 AP as float32r for full-rate PE matmuls (no-op for
    tiles already in a matmul dtype)."""
    if ap.dtype in (F32R, BF16):
        return ap
    return ap.bitcast(F32R)


def _bcast(ap_1d, parts):
    """Broadcast a 1-partition AP across `parts` partitions (step-0 AP)."""
    return bass.AP(tensor=ap_1d.tensor, offset=ap_1d.offset,
                   ap=[[0, parts]] + list(ap_1d.ap))


def _drain_until(stop_gen, *gens):
    """Round-robin stop_gen plus (gen, weight) extras; return when
    stop_gen exhausts (extras keep their state for later drains)."""
    live = [[g, w] for g, w in gens]
    while True:
        try:
            next(stop_gen)
        except StopIteration:
            return
        nxt = []
        for g, w in live:
            alive = True
            for _ in range(w):
                try:
                    next(g)
                except StopIteration:
                    alive = False
                    break
            if alive:
                nxt.append([g, w])
        live = nxt


def build_bass(reps=1):
    nc = bacc.Bacc(None, target_bir_lowering=False, debug=False)

    # ---- DRAM I/O ----
    x_in = nc.dram_tensor("x", [BPC, S, E], F32, kind="ExternalInput")
    qk_w = nc.dram_tensor("qk_w", [E, 512], BF16, kind="ExternalInput")
    qk_bc = nc.dram_tensor("qk_bc", [128, 4], F32, kind="ExternalInput")
    wv = nc.dram_tensor("wv", [E, H * (D + 1)], BF16, kind="ExternalInput")
    bv_row = nc.dram_tensor("bv_row", [1, H * (D + 1)], F32, kind="ExternalInput")
    out_w = nc.dram_tensor("out_w", [D, H, E], BF16, kind="ExternalInput")
    outb_row = nc.dram_tensor("outb_row", [1, E], F32, kind="ExternalInput")
    fc_w = nc.dram_tensor("fc_w", [E, FF], BF16, kind="ExternalInput")
    fc_bt = nc.dram_tensor("fc_bt", [128, FT], F32, kind="ExternalInput")
    proj_w = nc.dram_tensor("proj_w", [FF, E], BF16, kind="ExternalInput")
    projb_row = nc.dram_tensor("projb_row", [1, E], BF16, kind="ExternalInput")
    y_out = nc.dram_tensor("y", [BPC, S, E], F32, kind="ExternalOutput")

    VA = H * (D + 1)  # 260

    with tile.TileContext(nc) as tc:
        wp = tc.alloc_tile_pool(name="weights", bufs=1)
        sp = tc.alloc_tile_pool(name="small", bufs=2)
        bp2 = tc.alloc_tile_pool(name="big2", bufs=2)
        bp1 = tc.alloc_tile_pool(name="big1", bufs=1)
        psA = tc.alloc_tile_pool(name="psA", bufs=2, space="PSUM")
        psS = tc.alloc_tile_pool(name="psS", bufs=4, space="PSUM")
        psO = tc.alloc_tile_pool(name="psO", bufs=2, space="PSUM")

        # ---- persistent weights / constants ----
        qk_w_sb = wp.tile([128, ET, 512], BF16)
        nc.gpsimd.dma_start(out=qk_w_sb, in_=qk_w[:, :].rearrange("(t p) c -> p t c", p=128))
        qk_bc_sb = wp.tile([128, 4], F32)
        nc.gpsimd.dma_start(out=qk_bc_sb, in_=qk_bc[:, :])
        wv_sb = wp.tile([128, ET, VA], BF16)
        nc.gpsimd.dma_start(out=wv_sb, in_=wv[:, :].rearrange("(t p) c -> p t c", p=128))
        bv_sb = wp.tile([128, VA], F32)
        nc.gpsimd.dma_start(out=bv_sb, in_=_bcast(bv_row[0, :], 128))
        out_w_sb = wp.tile([64, H, E], BF16)
        nc.gpsimd.dma_start(out=out_w_sb, in_=out_w[:, :, :])
        outb_sb = wp.tile([128, E], F32)
        nc.gpsimd.dma_start(out=outb_sb, in_=_bcast(outb_row[0, :], 128))
        fc_w_sb = wp.tile([128, ET, FF], BF16)
        nc.gpsimd.dma_start(out=fc_w_sb, in_=fc_w[:, :].rearrange("(t p) c -> p t c", p=128))
        fc_bt_sb = wp.tile([128, FT], F32)
        nc.gpsimd.dma_start(out=fc_bt_sb, in_=fc_bt[:, :])
        proj_w_sb = wp.tile([128, FT, E], BF16)
        nc.gpsimd.dma_start(out=proj_w_sb, in_=proj_w[:, :].rearrange("(t p) c -> p t c", p=128))
        projb_sb = wp.tile([1, E], BF16)
        nc.gpsimd.dma_start(out=projb_sb, in_=projb_row[:, :])

        ones_row = wp.tile([1, 128], BF16)
        nc.vector.memset(ones_row, 1.0)
        ones_col = wp.tile([128, 64], BF16)
        nc.vector.memset(ones_col, 1.0)
        eps_col = wp.tile([128, 1], F32)
        nc.vector.memset(eps_col, EPS)
        ident = wp.tile([128, 128], F32)
        make_identity(nc, ident)
        # additive causal mask: NEG where q < k (strict lower in [k, q])
        negmask = wp.tile([128, 128], F32)
        make_lower_triangular(nc, negmask, val=NEG, diag=False)

        def ln_stats(src, t, stats, mv):
            nc.vector.bn_stats(out=stats[:, t, :], in_=src[:, t, :])
            nc.vector.bn_aggr(out=mv[:, t, :], in_=stats[:, t, :])

        def ln_coeffs(mv, rstd, nmr):
            # rstd = exp(-0.5 * ln(var + eps)); Ln/Exp share an ACT table set
            nc.scalar.activation(rstd, mv[:, :, 1], AF.Ln, bias=eps_col)
            nc.scalar.activation(rstd, rstd, AF.Exp, scale=-0.5)
            nc.vector.tensor_mul(nmr, mv[:, :, 0], rstd)
            nc.vector.tensor_scalar_mul(nmr, nmr, -1.0)

        def ln_apply(src, t, rstd, nmr, dst):
            nc.vector.tensor_scalar(
                out=dst, in0=src[:, t, :],
                scalar1=rstd[:, t:t + 1], scalar2=nmr[:, t:t + 1],
                op0=ALU.mult, op1=ALU.add)

        def gen_p1(b, xs, qkT, vv):
            """LN1 -> h1 -> h1T; qkT; vv.  Also xs += out_b (pre-bias for
            the out_proj residual) once LN1 has consumed raw xs."""
            for t in range(ST):
                nc.sync.dma_start(out=xs[:, t, :],
                                  in_=x_in[b, 128 * t:128 * (t + 1), :])
            stats = sp.tile([128, ST, 6], F32, tag="bnstats")
            mv = sp.tile([128, ST, 2], F32, tag="bnaggr")
            rstd = sp.tile([128, ST], F32, tag="rstd")
            nmr = sp.tile([128, ST], F32, tag="nmr")
            h1 = bp2.tile([128, ST, E], F32, tag="h1")
            h1T = bp2.tile([128, ET, S], BF16, tag="hT")
            # LN1 pipelined per 4-tile group: stats -> coeffs -> apply ->
            # transposes of the group, so the PE starts after 4 tiles
            for g in range(2):
                ts = range(4 * g, 4 * g + 4)
                for t in ts:
                    ln_stats(xs, t, stats, mv)
                yield
                gsl = slice(4 * g, 4 * g + 4)
                ln_coeffs(mv[:, gsl, :], rstd[:, gsl], nmr[:, gsl])
                for t in ts:
                    ln_apply(xs, t, rstd, nmr, h1[:, t, :])
                    # fold out_b into the residual (LN1 done with xs[t])
                    nc.gpsimd.tensor_add(xs[:, t, :], xs[:, t, :], outb_sb)
                yield
                for e in range(ET):
                    ps = psA.tile([128, 512], F32, tag="psA")
                    for i in range(4):
                        t = 4 * g + i
                        nc.tensor.transpose(
                            ps[:, 128 * i:128 * (i + 1)],
                            h1[:, t, 128 * e:128 * (e + 1)],
                            ident)
                    nc.scalar.copy(h1T[:, e, 512 * g:512 * (g + 1)], ps)
                    yield
            # qkT: m=0: q heads 0-1, m=1: q heads 2-3, m=2: k heads 0-1,
            # m=3: k heads 2-3 (rows = 2x64 head dims)
            for m in range(4):
                for c in range(2):
                    sl = slice(512 * c, 512 * (c + 1))
                    ps = psS.tile([128, 512], F32, tag="psS")
                    nc.tensor.matmul(ps, qk_w_sb[:, 0, 128 * m:128 * (m + 1)],
                                     h1T[:, 0, sl], start=True, stop=False)
                    nc.tensor.matmul(ps, qk_w_sb[:, 1, 128 * m:128 * (m + 1)],
                                     h1T[:, 1, sl], start=False, stop=True)
                    nc.vector.tensor_scalar(
                        out=qkT[:, m, sl], in0=ps, scalar1=qk_bc_sb[:, m:m + 1],
                        scalar2=None, op0=ALU.add)
                yield
            # v seq-major with per-head all-ones column: vv[:, t, 65h+64] == 1
            # (bias + the 1.0 column ride the DVE evacuation via bv_sb)
            for t in range(ST):
                ps = psA.tile([128, 512], F32, tag="psA")
                nc.tensor.matmul(ps[:, 0:VA], h1T[:, 0, 128 * t:128 * (t + 1)],
                                 wv_sb[:, 0, :], start=True, stop=False)
                nc.tensor.matmul(ps[:, 0:VA], h1T[:, 1, 128 * t:128 * (t + 1)],
                                 wv_sb[:, 1, :], start=False, stop=True)
                nc.vector.tensor_add(vv[:, t, :], ps[:, 0:VA], bv_sb)
                if t % 2 == 1:
                    yield

        def gen_p2(b, qkT, vv, oTh):
            """Attention: scoresT (+additive causal mask on PSUM) -> exp ->
            P@V_aug -> softmax normalization via a PE rank-1 broadcast of
            the sums row and a DVE divide (no DMA round trips)."""
            for pair in range(2):
                expTs = {}
                oscs = {}
                for hh in range(2):
                    h = 2 * pair + hh
                    expTs[h] = bp2.tile([128, ETOT], BF16, tag="expT",
                                        name=f"expT{h}", bufs=4)
                    oscs[h] = sp.tile([65, 2, 512], BF16, tag="osc",
                                      name=f"osc{h}", bufs=4)

                def scores_j(j):
                    w0 = 128 * j
                    pss = {}
                    for hh in range(2):
                        h = 2 * pair + hh
                        qsl = slice(64 * hh, 64 * hh + 64)
                        for c in range(w0 // 512, 2):
                            a = max(w0, 512 * c)
                            ps = psS.tile([128, 512], F32, tag="psS",
                                          name=f"psS{h}c{c}")
                            pss[(h, c)] = ps
                            nc.tensor.matmul(
                                ps[:, a - 512 * c:512],
                                qkT[qsl, 2 + pair, w0:w0 + 128],
                                qkT[qsl, pair, a:512 * (c + 1)],
                                start=True, stop=True)
                            if a == w0:
                                # additive causal mask on the diagonal block
                                nc.vector.tensor_add(
                                    ps[:, a - 512 * c:a - 512 * c + 128],
                                    ps[:, a - 512 * c:a - 512 * c + 128],
                                    negmask)
                    for c in range(w0 // 512, 2):
                        a = max(w0, 512 * c)
                        for hh in range(2):
                            h = 2 * pair + hh
                            nc.scalar.activation(
                                expTs[h][:, EOFF[j] + a - w0:
                                         EOFF[j] + 512 * (c + 1) - w0],
                                pss[(h, c)][:, a - 512 * c:512],
                                AF.Exp, scale=0.125)

                def attv(hh, c):
                    h = 2 * pair + hh
                    expT = expTs[h]
                    po = psO.tile([128, 512], F32, tag="psO")
                    jmax = 4 * (c + 1)
                    for j in range(jmax):
                        a = max(128 * j, 512 * c)
                        nc.tensor.matmul(
                            po[0:65, a - 512 * c:512],
                            vv[:, j, 65 * h:65 * (h + 1)],
                            expT[:, EOFF[j] + a - 128 * j:
                                 EOFF[j] + 512 * (c + 1) - 128 * j],
                            start=(j == 0), stop=(j == jmax - 1))
                    # unnormalized o rows + denominator row -> SBUF scratch
                    nc.vector.tensor_copy(oscs[h][:, c, :], po[0:65, :])
                    # broadcast the sums row over 64 partitions on the PE
                    # (rank-1 ones_col x sums_row), then normalize with a
                    # DVE divide -- no DMA round trip
                    sb = psO.tile([128, 512], F32, tag="psO", name="sb")
                    nc.tensor.matmul(sb[0:64, :], ones_col[64:65, :],
                                     oscs[h][64:65, c, :],
                                     start=True, stop=True)
                    rb = sp.tile([64, 512], F32, tag="rb", bufs=4)
                    nc.vector.reciprocal_approx_fast(out=rb, in_=sb[0:64, :])
                    eng = nc.vector if hh == 0 else nc.gpsimd
                    eng.tensor_mul(
                        oTh[h][:, 512 * c:512 * (c + 1)],
                        oscs[h][0:64, c, :], rb)

                for j in range(4):
                    scores_j(j)
                    yield
                for hh in range(2):
                    attv(hh, 0)
                    yield
                for j in range(4, ST):
                    scores_j(j)
                    yield
                for hh in range(2):
                    attv(hh, 1)
                    yield

        def gen_p3a(b, xs, oTh, x2, st3):
            """out_proj + residual -> x2; LN2 -> h2 -> h2T (stored into
            st3['h2T'] for gen_p3b)."""
            stats = sp.tile([128, ST, 6], F32, tag="bnstats")
            mv = sp.tile([128, ST, 2], F32, tag="bnaggr")
            for t in range(ST):
                ps = psA.tile([128, 512], F32, tag="psA")
                for h in range(H):
                    nc.tensor.matmul(ps[:, 0:E],
                                     oTh[h][:, 128 * t:128 * (t + 1)],
                                     out_w_sb[:, h, :],
                                     start=(h == 0), stop=(h == H - 1))
                # xs already carries +out_b
                nc.vector.tensor_add(x2[:, t, :], ps[:, 0:E], xs[:, t, :])
                ln_stats(x2, t, stats, mv)
                if t % 2 == 1:
                    yield
            rstd = sp.tile([128, ST], F32, tag="rstd")
            nmr = sp.tile([128, ST], F32, tag="nmr")
            h2 = bp2.tile([128, ST, E], F32, tag="h1")
            h2T = bp2.tile([128, ET, S], BF16, tag="hT")
            st3["h2T"] = h2T
            for g in range(2):
                gsl = slice(4 * g, 4 * g + 4)
                ln_coeffs(mv[:, gsl, :], rstd[:, gsl], nmr[:, gsl])
                for t in range(4 * g, 4 * g + 4):
                    ln_apply(x2, t, rstd, nmr, h2[:, t, :])
                yield
                for e in range(ET):
                    ps = psA.tile([128, 512], F32, tag="psA")
                    for i in range(4):
                        t = 4 * g + i
                        nc.tensor.transpose(
                            ps[:, 128 * i:128 * (i + 1)],
                            h2[:, t, 128 * e:128 * (e + 1)],
                            ident)
                    nc.scalar.copy(h2T[:, e, 512 * g:512 * (g + 1)], ps)
                    yield

        def gen_p3b(b, xs, x2, st3):
            """MLP fc + relu -> uT; proj + residual -> y (reuses xs)."""
            h2T = st3["h2T"]
            # ---- MLP fc + relu (uT feature-major) ----
            uT = bp1.tile([128, FT, S], BF16, tag="uT")
            for m in range(FT):
                for c in range(2):
                    sl = slice(512 * c, 512 * (c + 1))
                    ps = psS.tile([128, 512], F32, tag="psS")
                    nc.tensor.matmul(ps, fc_w_sb[:, 0, 128 * m:128 * (m + 1)],
                                     h2T[:, 0, sl], start=True, stop=False)
                    nc.tensor.matmul(ps, fc_w_sb[:, 1, 128 * m:128 * (m + 1)],
                                     h2T[:, 1, sl], start=False, stop=True)
                    if c == 0:
                        nc.scalar.activation(uT[:, m, sl], ps, AF.Relu,
                                             bias=fc_bt_sb[:, m:m + 1])
                    else:
                        nc.vector.tensor_scalar(
                            out=uT[:, m, sl], in0=ps,
                            scalar1=fc_bt_sb[:, m:m + 1], scalar2=0.0,
                            op0=ALU.add, op1=ALU.max)
                yield
            # ---- proj + residual -> y (reuses the xs tile in place) ----
            for t in range(ST):
                ps = psA.tile([128, 512], F32, tag="psA")
                for m in range(FT):
                    nc.tensor.matmul(ps[:, 0:E], uT[:, m, 128 * t:128 * (t + 1)],
                                     proj_w_sb[:, m, :],
                                     start=(m == 0), stop=False)
                nc.tensor.matmul(ps[:, 0:E], ones_row,
                                 projb_sb[0:1, :], start=False, stop=True)
                nc.vector.tensor_add(xs[:, t, :], ps[:, 0:E], x2[:, t, :])
                nc.sync.dma_start(out=y_out[b, 128 * t:128 * (t + 1), :],
                                  in_=xs[:, t, :])
                yield

        for _rep in range(reps):
            xs_t, qkT_t, vv_t, oTh_t, x2_t = [], [], [], [], []
            for b in range(BPC):
                xs_t.append(bp2.tile([128, ST, E], F32, tag="xs",
                                     name=f"xs{b}"))
                qkT_t.append(bp1.tile([128, 4, S], BF16, tag="qkT",
                                      name=f"qkT{b}", bufs=2))
                vv_t.append(bp1.tile([128, ST, VA], BF16, tag="vv",
                                     name=f"vv{b}", bufs=2))
                oTh_t.append([bp1.tile([64, S], BF16, tag="oTh",
                                       name=f"oTh{b}_{h}", bufs=8)
                              for h in range(H)])
                x2_t.append(bp1.tile([128, ST, E], F32, tag="x2",
                                     name=f"x2{b}", bufs=2))
            st3 = [{}, {}]

            g10 = gen_p1(0, xs_t[0], qkT_t[0], vv_t[0])
            g11 = gen_p1(1, xs_t[1], qkT_t[1], vv_t[1])
            g20 = gen_p2(0, qkT_t[0], vv_t[0], oTh_t[0])
            g21 = gen_p2(1, qkT_t[1], vv_t[1], oTh_t[1])
            g30a = gen_p3a(0, xs_t[0], oTh_t[0], x2_t[0], st3[0])
            g30b = gen_p3b(0, xs_t[0], x2_t[0], st3[0])
            g31a = gen_p3a(1, xs_t[1], oTh_t[1], x2_t[1], st3[1])
            g31b = gen_p3b(1, xs_t[1], x2_t[1], st3[1])

            # Chained emission schedule: each drain stops when its leader
            # exhausts; partners carry over with their state.  A partner may
            # only join once everything it depends on is fully emitted
            # (in-order engine queues would deadlock otherwise), so p1 runs
            # to completion for both batches before p2(1) joins, and the
            # MLP (psS/uT user) never overlaps attention of the same or
            # other batch.
            _drain_until(g10, (g11, 1))
            _drain_until(g11)
            _drain_until(g20, (g21, 1))
            _drain_until(g21, (g30a, 1))
            _drain_until(g30a)
            _drain_until(g30b, (g31a, 1))
            _drain_until(g31a)
            _drain_until(g31b)

        for p in (psO, psS, psA, bp1, bp2, sp, wp):
            p.release()

    nc.compile()
    return nc


def host_prep(inputs):
    """Fold LN params into weights; build the DRAM-side weight layouts."""
    f = np.float32
    qkv_w = np.asarray(inputs["qkv_w"], f)
    qkv_b = np.asarray(inputs["qkv_b"], f)
    ln1_g = np.asarray(inputs["ln1_g"], f)
    ln1_b = np.asarray(inputs["ln1_b"], f)
    ln2_g = np.asarray(inputs["ln2_g"], f)
    ln2_b = np.asarray(inputs["ln2_b"], f)
    fc_w = np.asarray(inputs["fc_w"], f)
    fc_b = np.asarray(inputs["fc_b"], f)

    W1 = qkv_w * ln1_g[:, None]
    b1 = qkv_b + ln1_b @ qkv_w
    W2 = fc_w * ln2_g[:, None]
    b2 = fc_b + ln2_b @ fc_w

    qk_w = np.ascontiguousarray(W1[:, :512])
    qk_bc = np.ascontiguousarray(b1[:512].reshape(4, 128).T)
    wv = np.zeros((E, H * (D + 1)), f)
    bv = np.zeros((1, H * (D + 1)), f)
    for h in range(H):
        wv[:, 65 * h:65 * h + 64] = W1[:, 512 + 64 * h:512 + 64 * (h + 1)]
        bv[0, 65 * h:65 * h + 64] = b1[512 + 64 * h:512 + 64 * (h + 1)]
        bv[0, 65 * h + 64] = 1.0
    fc_bt = np.ascontiguousarray(b2.reshape(FT, 128).T)

    import ml_dtypes
    bf = ml_dtypes.bfloat16

    return {
        "qk_w": qk_w.astype(bf),
        "qk_bc": qk_bc,
        "wv": wv.astype(bf),
        "bv_row": bv,
        "out_w": np.ascontiguousarray(
            np.asarray(inputs["out_w"], f).reshape(H, D, E)
            .transpose(1, 0, 2)).astype(bf),
        "outb_row": np.asarray(inputs["out_b"], f).reshape(1, E),
        "fc_w": W2.astype(bf),
        "fc_bt": fc_bt,
        "proj_w": np.asarray(inputs["proj_w"], f).astype(bf),
        "projb_row": np.asarray(inputs["proj_b"], f).reshape(1, E).astype(bf),
    }


_NC_CACHE = None


def _get_nc():
    global _NC_CACHE
    if _NC_CACHE is None:
        _NC_CACHE = build_bass()
    return _NC_CACHE


def run(inputs, trace=False):
    from concourse.bass_utils import run_bass_kernel_spmd

    nc = _get_nc()
    weights = host_prep(inputs)
    x = np.asarray(inputs["x"], np.float32)
    in_maps = []
    for c in range(NCORES):
        m = dict(weights)
        m["x"] = np.ascontiguousarray(x[BPC * c:BPC * (c + 1)])
        in_maps.append(m)
    res = run_bass_kernel_spmd(nc, in_maps, core_ids=list(range(NCORES)),
                               trace=trace)
    y = np.concatenate([res.results[c]["y"] for c in range(NCORES)], axis=0)
    return y, res


def kernel(**inputs):
    y, _ = run(inputs)
    return y


# revision 14
# speedup vs baseline: 1.1110x; 1.1110x over previous
"""Trainium2 Bass kernel for a GPT-style transformer block.

Reference computation (per batch element):
    h  = LN1(x);  qkv = h @ qkv_w + qkv_b
    att = causal_softmax(q @ k.T / sqrt(64));  o = att @ v
    x  = x + o @ out_w + out_b
    h  = LN2(x);  u = relu(h @ fc_w + fc_b)
    y  = x + u @ proj_w + proj_b

Shapes: x [16, 1024, 256], 4 heads x 64, MLP hidden 1024.

Strategy: pure data-parallel over batch, 2 batch elements per core on 8
cores, no collectives.  Within a core:
  - LN gamma/beta folded into the following matmul weights on the host.
  - The residual stream (xs/x2) and LN math stay fp32; matmul operands
    (h1T/qkT/vv/expT/uT + weights) are bf16 except out_proj (f32r).
  - Attention scores are computed transposed (scoresT[k, q]) so the
    causal region is per-k-tile a contiguous q-range.  The causal mask is
    ADDITIVE (-1e30) applied by DVE on the PSUM diagonal block before the
    scalar-engine exp, so P@V waits only on exps.  Softmax denominators
    come from an extra all-ones column appended to V (P@V_aug also yields
    row sums); normalization uses a sums -> reciprocal DRAM round trip
    that lands per-partition reciprocal columns broadcast over the o rows.
  - Biases ride evacuation ops (DVE adds against broadcast bias tiles,
    out_b pre-added into the residual) instead of rank-1 PE matmuls
    (except proj_b, whose residual target also feeds LN2).
  - Emission is generator-driven and interleaved so each engine's
    in-order queue rarely has a blocked op ahead of ready work:
    p1(0)+p1(1) | p2(0)+p2(1) | p2(1)+p3(0) | p3(0)+p3(1) | p3(1).
"""

import sys

sys.path.insert(0, "/opt/trn_rl_repo")

import numpy as np

import concourse.bass as bass
import concourse.bacc as bacc
import concourse.tile as tile
from concourse import mybir
from concourse.masks import make_identity, make_lower_triangular

# The activation-set chooser is first-match, which thrashes table loads
# between the exp and ln sets.  natural_log_exp_and_others contains every
# function this kernel uses (exp, ln, copy/identity, relu), so restrict
# the chooser to it (other entries stay in place so set ids keep their
# positions for walrus).
_orig_get_act_tables = bacc.get_activation_tables


def _one_set_tables(module_arch):
    tabs = _orig_get_act_tables(module_arch)
    return {name: (fns if name == "natural_log_exp_and_others" else set())
            for name, fns in tabs.items()}


bacc.get_activation_tables = _one_set_tables

F32 = mybir.dt.float32
F32R = mybir.dt.float32r
BF16 = mybir.dt.bfloat16
AF = mybir.ActivationFunctionType
ALU = mybir.AluOpType

NCORES = 8
B = 16
BPC = B // NCORES  # 2 batch elements per core
S = 1024
E = 256
H = 4
D = 64
FF = 1024
ST = S // 128  # 8 seq tiles
ET = E // 128  # 2 feature tiles
FT = FF // 128  # 8 mlp-hidden tiles
EPS = 1e-5
NEG = -1.0e30

# ragged offsets for the causal expT store: tile j holds q in [128j, S)
EOFF = [0]
for _j in range(ST):
    EOFF.append(EOFF[-1] + (S - 128 * _j))
ETOT = EOFF[ST]  # 4608


def _r(ap):
    """View an fp32 AP as float32r for full-rate PE matmuls (no-op for
    tiles already in a matmul dtype)."""
    if ap.dtype in (F32R, BF16):
        return ap
    return ap.bitcast(F32R)


def _bcast(ap_1d, parts):
    """Broadcast a 1-partition AP across `parts` partitions (step-0 AP)."""
    return bass.AP(tensor=ap_1d.tensor, offset=ap_1d.offset,
                   ap=[[0, parts]] + list(ap_1d.ap))


def _drain_until(stop_gen, *gens):
    """Round-robin stop_gen plus (gen, weight) extras; return when
    stop_gen exhausts (extras keep their state for later drains)."""
    live = [[g, w] for g, w in gens]
    while True:
        try:
            next(stop_gen)
        except StopIteration:
            return
        nxt = []
        for g, w in live:
            alive = True
            for _ in range(w):
                try:
                    next(g)
                except StopIteration:
                    alive = False
                    break
            if alive:
                nxt.append([g, w])
        live = nxt


def build_bass(reps=1):
    nc = bacc.Bacc(None, target_bir_lowering=False, debug=False)

    # ---- DRAM I/O ----
    x_in = nc.dram_tensor("x", [BPC, S, E], F32, kind="ExternalInput")
    qk_w = nc.dram_tensor("qk_w", [E, 512], BF16, kind="ExternalInput")
    qk_bc = nc.dram_tensor("qk_bc", [128, 4], F32, kind="ExternalInput")
    wv = nc.dram_tensor("wv", [E, H * (D + 1)], BF16, kind="ExternalInput")
    bv_row = nc.dram_tensor("bv_row", [1, H * (D + 1)], F32, kind="ExternalInput")
    out_w = nc.dram_tensor("out_w", [D, H, E], BF16, kind="ExternalInput")
    outb_row = nc.dram_tensor("outb_row", [1, E], F32, kind="ExternalInput")
    fc_w = nc.dram_tensor("fc_w", [E, FF], BF16, kind="ExternalInput")
    fc_bt = nc.dram_tensor("fc_bt", [128, FT], F32, kind="ExternalInput")
    proj_w = nc.dram_tensor("proj_w", [FF, E], BF16, kind="ExternalInput")
    projb_row = nc.dram_tensor("projb_row", [1, E], BF16, kind="ExternalInput")
    y_out = nc.dram_tensor("y", [BPC, S, E], F32, kind="ExternalOutput")

    VA = H * (D + 1)  # 260

    with tile.TileContext(nc) as tc:
        wp = tc.alloc_tile_pool(name="weights", bufs=1)
        sp = tc.alloc_tile_pool(name="small", bufs=2)
        bp2 = tc.alloc_tile_pool(name="big2", bufs=2)
        bp1 = tc.alloc_tile_pool(name="big1", bufs=1)
        psA = tc.alloc_tile_pool(name="psA", bufs=2, space="PSUM")
        psS = tc.alloc_tile_pool(name="psS", bufs=4, space="PSUM")
        psO = tc.alloc_tile_pool(name="psO", bufs=2, space="PSUM")

        # ---- persistent weights / constants ----
        qk_w_sb = wp.tile([128, ET, 512], BF16)
        nc.gpsimd.dma_start(out=qk_w_sb, in_=qk_w[:, :].rearrange("(t p) c -> p t c", p=128))
        qk_bc_sb = wp.tile([128, 4], F32)
        nc.gpsimd.dma_start(out=qk_bc_sb, in_=qk_bc[:, :])
        wv_sb = wp.tile([128, ET, VA], BF16)
        nc.gpsimd.dma_start(out=wv_sb, in_=wv[:, :].rearrange("(t p) c -> p t c", p=128))
        bv_sb = wp.tile([128, VA], F32)
        nc.gpsimd.dma_start(out=bv_sb, in_=_bcast(bv_row[0, :], 128))
        out_w_sb = wp.tile([64, H, E], BF16)
        nc.gpsimd.dma_start(out=out_w_sb, in_=out_w[:, :, :])
        outb_sb = wp.tile([128, E], F32)
        nc.gpsimd.dma_start(out=outb_sb, in_=_bcast(outb_row[0, :], 128))
        fc_w_sb = wp.tile([128, ET, FF], BF16)
        nc.gpsimd.dma_start(out=fc_w_sb, in_=fc_w[:, :].rearrange("(t p) c -> p t c", p=128))
        fc_bt_sb = wp.tile([128, FT], F32)
        nc.gpsimd.dma_start(out=fc_bt_sb, in_=fc_bt[:, :])
        proj_w_sb = wp.tile([128, FT, E], BF16)
        nc.gpsimd.dma_start(out=proj_w_sb, in_=proj_w[:, :].rearrange("(t p) c -> p t c", p=128))
        projb_sb = wp.tile([1, E], BF16)
        nc.gpsimd.dma_start(out=projb_sb, in_=projb_row[:, :])

        ones_row = wp.tile([1, 128], BF16)
        nc.vector.memset(ones_row, 1.0)
        ones_col = wp.tile([128, 64], BF16)
        nc.vector.memset(ones_col, 1.0)
        eps_col = wp.tile([128, 1], F32)
        nc.vector.memset(eps_col, EPS)
        ident = wp.tile([128, 128], F32)
        make_identity(nc, ident)
        # additive causal mask: NEG where q < k (strict lower in [k, q])
        negmask = wp.tile([128, 128], F32)
        make_lower_triangular(nc, negmask, val=NEG, diag=False)

        def ln_stats(src, t, stats, mv):
            nc.vector.bn_stats(out=stats[:, t, :], in_=src[:, t, :])
            nc.vector.bn_aggr(out=mv[:, t, :], in_=stats[:, t, :])

        def ln_coeffs(mv, rstd, nmr):
            # rstd = exp(-0.5 * ln(var + eps)); Ln/Exp share an ACT table set
            nc.scalar.activation(rstd, mv[:, :, 1], AF.Ln, bias=eps_col)
            nc.scalar.activation(rstd, rstd, AF.Exp, scale=-0.5)
            nc.vector.tensor_mul(nmr, mv[:, :, 0], rstd)
            nc.vector.tensor_scalar_mul(nmr, nmr, -1.0)

        def ln_apply(src, t, rstd, nmr, dst):
            nc.vector.tensor_scalar(
                out=dst, in0=src[:, t, :],
                scalar1=rstd[:, t:t + 1], scalar2=nmr[:, t:t + 1],
                op0=ALU.mult, op1=ALU.add)

        def gen_p1(b, xs, qkT, vv):
            """LN1 -> h1 -> h1T; qkT; vv.  Also xs += out_b (pre-bias for
            the out_proj residual) once LN1 has consumed raw xs."""
            for t in range(ST):
                nc.sync.dma_start(out=xs[:, t, :],
                                  in_=x_in[b, 128 * t:128 * (t + 1), :])
            stats = sp.tile([128, ST, 6], F32, tag="bnstats")
            mv = sp.tile([128, ST, 2], F32, tag="bnaggr")
            rstd = sp.tile([128, ST], F32, tag="rstd")
            nmr = sp.tile([128, ST], F32, tag="nmr")
            h1 = bp2.tile([128, ST, E], F32, tag="h1")
            h1T = bp2.tile([128, ET, S], BF16, tag="hT")
            # LN1 pipelined per 4-tile group: stats -> coeffs -> apply ->
            # transposes of the group, so the PE starts after 4 tiles
            for g in range(2):
                ts = range(4 * g, 4 * g + 4)
                for t in ts:
                    ln_stats(xs, t, stats, mv)
                yield
                gsl = slice(4 * g, 4 * g + 4)
                ln_coeffs(mv[:, gsl, :], rstd[:, gsl], nmr[:, gsl])
                for t in ts:
                    ln_apply(xs, t, rstd, nmr, h1[:, t, :])
                    # fold out_b into the residual (LN1 done with xs[t])
                    nc.gpsimd.tensor_add(xs[:, t, :], xs[:, t, :], outb_sb)
                yield
                for e in range(ET):
                    ps = psA.tile([128, 512], F32, tag="psA")
                    for i in range(4):
                        t = 4 * g + i
                        nc.tensor.transpose(
                            ps[:, 128 * i:128 * (i + 1)],
                            h1[:, t, 128 * e:128 * (e + 1)],
                            ident)
                    nc.scalar.copy(h1T[:, e, 512 * g:512 * (g + 1)], ps)
                    yield
            # qkT: m=0: q heads 0-1, m=1: q heads 2-3, m=2: k heads 0-1,
            # m=3: k heads 2-3 (rows = 2x64 head dims)
            for m in range(4):
                for c in range(2):
                    sl = slice(512 * c, 512 * (c + 1))
                    ps = psS.tile([128, 512], F32, tag="psS")
                    nc.tensor.matmul(ps, qk_w_sb[:, 0, 128 * m:128 * (m + 1)],
                                     h1T[:, 0, sl], start=True, stop=False)
                    nc.tensor.matmul(ps, qk_w_sb[:, 1, 128 * m:128 * (m + 1)],
                                     h1T[:, 1, sl], start=False, stop=True)
                    nc.vector.tensor_scalar(
                        out=qkT[:, m, sl], in0=ps, scalar1=qk_bc_sb[:, m:m + 1],
                        scalar2=None, op0=ALU.add)
                yield
            # v seq-major with per-head all-ones column: vv[:, t, 65h+64] == 1
            # (bias + the 1.0 column ride the DVE evacuation via bv_sb)
            for t in range(ST):
                ps = psA.tile([128, 512], F32, tag="psA")
                nc.tensor.matmul(ps[:, 0:VA], h1T[:, 0, 128 * t:128 * (t + 1)],
                                 wv_sb[:, 0, :], start=True, stop=False)
                nc.tensor.matmul(ps[:, 0:VA], h1T[:, 1, 128 * t:128 * (t + 1)],
                                 wv_sb[:, 1, :], start=False, stop=True)
                nc.vector.tensor_add(vv[:, t, :], ps[:, 0:VA], bv_sb)
                if t % 2 == 1:
                    yield

        def gen_p2(b, qkT, vv, oTh):
            """Attention: scoresT (+additive causal mask on PSUM) -> exp ->
            P@V_aug -> softmax normalization via a PE rank-1 broadcast of
            the sums row and a DVE divide (no DMA round trips)."""
            for pair in range(2):
                expTs = {}
                oscs = {}
                for hh in range(2):
                    h = 2 * pair + hh
                    expTs[h] = bp2.tile([128, ETOT], BF16, tag="expT",
                                        name=f"expT{h}", bufs=4)
                    oscs[h] = sp.tile([65, 2, 512], BF16, tag="osc",
                                      name=f"osc{h}", bufs=4)

                def scores_j(j):
                    w0 = 128 * j
                    pss = {}
                    for hh in range(2):
                        h = 2 * pair + hh
                        qsl = slice(64 * hh, 64 * hh + 64)
                        for c in range(w0 // 512, 2):
                            a = max(w0, 512 * c)
                            ps = psS.tile([128, 512], F32, tag="psS",
                                          name=f"psS{h}c{c}")
                            pss[(h, c)] = ps
                            nc.tensor.matmul(
                                ps[:, a - 512 * c:512],
                                qkT[qsl, 2 + pair, w0:w0 + 128],
                                qkT[qsl, pair, a:512 * (c + 1)],
                                start=True, stop=True)
                            if a == w0:
                                # additive causal mask on the diagonal block
                                nc.vector.tensor_add(
                                    ps[:, a - 512 * c:a - 512 * c + 128],
                                    ps[:, a - 512 * c:a - 512 * c + 128],
                                    negmask)
                    for c in range(w0 // 512, 2):
                        a = max(w0, 512 * c)
                        for hh in range(2):
                            h = 2 * pair + hh
                            nc.scalar.activation(
                                expTs[h][:, EOFF[j] + a - w0:
                                         EOFF[j] + 512 * (c + 1) - w0],
                                pss[(h, c)][:, a - 512 * c:512],
                                AF.Exp, scale=0.125)

                def attv(hh, c):
                    h = 2 * pair + hh
                    expT = expTs[h]
                    po = psO.tile([128, 512], F32, tag="psO")
                    jmax = 4 * (c + 1)
                    for j in range(jmax):
                        a = max(128 * j, 512 * c)
                        nc.tensor.matmul(
                            po[0:65, a - 512 * c:512],
                            vv[:, j, 65 * h:65 * (h + 1)],
                            expT[:, EOFF[j] + a - 128 * j:
                                 EOFF[j] + 512 * (c + 1) - 128 * j],
                            start=(j == 0), stop=(j == jmax - 1))
                    # unnormalized o rows + denominator row -> SBUF scratch
                    nc.vector.tensor_copy(oscs[h][:, c, :], po[0:65, :])
                    # broadcast the sums row over 64 partitions on the PE
                    # (rank-1 ones_col x sums_row), then normalize with a
                    # DVE divide -- no DMA round trip
                    sb = psO.tile([128, 512], F32, tag="psO", name="sb")
                    nc.tensor.matmul(sb[0:64, :], ones_col[64:65, :],
                                     oscs[h][64:65, c, :],
                                     start=True, stop=True)
                    rb = sp.tile([64, 512], F32, tag="rb", bufs=4)
                    nc.vector.reciprocal_approx_fast(out=rb, in_=sb[0:64, :])
                    eng = nc.vector if hh == 0 else nc.gpsimd
                    eng.tensor_mul(
                        oTh[h][:, 512 * c:512 * (c + 1)],
                        oscs[h][0:64, c, :], rb)

                for j in range(4):
                    scores_j(j)
                    yield
                for hh in range(2):
                    attv(hh, 0)
                    yield
                for j in range(4, ST):
                    scores_j(j)
                    yield
                for hh in range(2):
                    attv(hh, 1)
                    yield

        def gen_p3a(b, xs, oTh, x2, st3):
            """out_proj + residual -> x2; LN2 -> h2 -> h2T (stored into
            st3['h2T'] for gen_p3b)."""
            stats = sp.tile([128, ST, 6], F32, tag="bnstats")
            mv = sp.tile([128, ST, 2], F32, tag="bnaggr")
            for t in range(ST):
                ps = psA.tile([128, 512], F32, tag="psA")
                for h in range(H):
                    nc.tensor.matmul(ps[:, 0:E],
                                     oTh[h][:, 128 * t:128 * (t + 1)],
                                     out_w_sb[:, h, :],
                                     start=(h == 0), stop=(h == H - 1))
                # xs already carries +out_b
                nc.vector.tensor_add(x2[:, t, :], ps[:, 0:E], xs[:, t, :])
                ln_stats(x2, t, stats, mv)
                if t % 2 == 1:
                    yield
            rstd = sp.tile([128, ST], F32, tag="rstd")
            nmr = sp.tile([128, ST], F32, tag="nmr")
            h2 = bp2.tile([128, ST, E], F32, tag="h1")
            h2T = bp2.tile([128, ET, S], BF16, tag="hT")
            st3["h2T"] = h2T
            for g in range(2):
                gsl = slice(4 * g, 4 * g + 4)
                ln_coeffs(mv[:, gsl, :], rstd[:, gsl], nmr[:, gsl])
                for t in range(4 * g, 4 * g + 4):
                    ln_apply(x2, t, rstd, nmr, h2[:, t, :])
                yield
                for e in range(ET):
                    ps = psA.tile([128, 512], F32, tag="psA")
                    for i in range(4):
                        t = 4 * g + i
                        nc.tensor.transpose(
                            ps[:, 128 * i:128 * (i + 1)],
                            h2[:, t, 128 * e:128 * (e + 1)],
                            ident)
                    nc.scalar.copy(h2T[:, e, 512 * g:512 * (g + 1)], ps)
                    yield

        def gen_p3b(b, xs, x2, st3):
            """MLP fc + relu -> uT; proj + residual -> y (reuses xs)."""
            h2T = st3["h2T"]
            # ---- MLP fc + relu (uT feature-major) ----
            uT = bp1.tile([128, FT, S], BF16, tag="uT")
            for m in range(FT):
                for c in range(2):
                    sl = slice(512 * c, 512 * (c + 1))
                    ps = psA.tile([128, 512], F32, tag="psA")
                    nc.tensor.matmul(ps, fc_w_sb[:, 0, 128 * m:128 * (m + 1)],
                                     h2T[:, 0, sl], start=True, stop=False)
                    nc.tensor.matmul(ps, fc_w_sb[:, 1, 128 * m:128 * (m + 1)],
                                     h2T[:, 1, sl], start=False, stop=True)
                    if c == 0:
                        nc.scalar.activation(uT[:, m, sl], ps, AF.Relu,
                                             bias=fc_bt_sb[:, m:m + 1])
                    else:
                        nc.vector.tensor_scalar(
                            out=uT[:, m, sl], in0=ps,
                            scalar1=fc_bt_sb[:, m:m + 1], scalar2=0.0,
                            op0=ALU.add, op1=ALU.max)
                yield
            # ---- proj + residual -> y (reuses the xs tile in place) ----
            for t in range(ST):
                ps = psA.tile([128, 512], F32, tag="psA")
                for m in range(FT):
                    nc.tensor.matmul(ps[:, 0:E], uT[:, m, 128 * t:128 * (t + 1)],
                                     proj_w_sb[:, m, :],
                                     start=(m == 0), stop=False)
                nc.tensor.matmul(ps[:, 0:E], ones_row,
                                 projb_sb[0:1, :], start=False, stop=True)
                nc.vector.tensor_add(xs[:, t, :], ps[:, 0:E], x2[:, t, :])
                nc.sync.dma_start(out=y_out[b, 128 * t:128 * (t + 1), :],
                                  in_=xs[:, t, :])
                yield

        for _rep in range(reps):
            xs_t, qkT_t, vv_t, oTh_t, x2_t = [], [], [], [], []
            for b in range(BPC):
                xs_t.append(bp2.tile([128, ST, E], F32, tag="xs",
                                     name=f"xs{b}"))
                qkT_t.append(bp1.tile([128, 4, S], BF16, tag="qkT",
                                      name=f"qkT{b}", bufs=2))
                vv_t.append(bp1.tile([128, ST, VA], BF16, tag="vv",
                                     name=f"vv{b}", bufs=2))
                oTh_t.append([bp1.tile([64, S], BF16, tag="oTh",
                                       name=f"oTh{b}_{h}", bufs=8)
                              for h in range(H)])
                x2_t.append(bp1.tile([128, ST, E], F32, tag="x2",
                                     name=f"x2{b}", bufs=2))
            st3 = [{}, {}]

            g10 = gen_p1(0, xs_t[0], qkT_t[0], vv_t[0])
            g11 = gen_p1(1, xs_t[1], qkT_t[1], vv_t[1])
            g20 = gen_p2(0, qkT_t[0], vv_t[0], oTh_t[0])
            g21 = gen_p2(1, qkT_t[1], vv_t[1], oTh_t[1])
            g30a = gen_p3a(0, xs_t[0], oTh_t[0], x2_t[0], st3[0])
            g30b = gen_p3b(0, xs_t[0], x2_t[0], st3[0])
            g31a = gen_p3a(1, xs_t[1], oTh_t[1], x2_t[1], st3[1])
            g31b = gen_p3b(1, xs_t[1], x2_t[1], st3[1])

            def _chain(*gens):
                for g in gens:
                    yield from g

            g30 = _chain(g30a, g30b)
            g31 = _chain(g31a, g31b)

            # Chained emission schedule: each drain stops when its leader
            # exhausts; partners carry over with their state.  Attention of
            # one batch is co-scheduled with the PE-dense p1/p3 work of the
            # OTHER batch so the PE stays continuously busy (p-state) while
            # the scalar engine chews exps.
            _drain_until(g10, (g11, 1))
            _drain_until(g20, (g11, 1))
            _drain_until(g21, (g30, 1))
            _drain_until(g30)
            _drain_until(g31)

        for p in (psO, psS, psA, bp1, bp2, sp, wp):
            p.release()

    nc.compile()
    return nc


def host_prep(inputs):
    """Fold LN params into weights; build the DRAM-side weight layouts."""
    f = np.float32
    qkv_w = np.asarray(inputs["qkv_w"], f)
    qkv_b = np.asarray(inputs["qkv_b"], f)
    ln1_g = np.asarray(inputs["ln1_g"], f)
    ln1_b = np.asarray(inputs["ln1_b"], f)
    ln2_g = np.asarray(inputs["ln2_g"], f)
    ln2_b = np.asarray(inputs["ln2_b"], f)
    fc_w = np.asarray(inputs["fc_w"], f)
    fc_b = np.asarray(inputs["fc_b"], f)

    W1 = qkv_w * ln1_g[:, None]
    b1 = qkv_b + ln1_b @ qkv_w
    W2 = fc_w * ln2_g[:, None]
    b2 = fc_b + ln2_b @ fc_w

    qk_w = np.ascontiguousarray(W1[:, :512])
    qk_bc = np.ascontiguousarray(b1[:512].reshape(4, 128).T)
    wv = np.zeros((E, H * (D + 1)), f)
    bv = np.zeros((1, H * (D + 1)), f)
    for h in range(H):
        wv[:, 65 * h:65 * h + 64] = W1[:, 512 + 64 * h:512 + 64 * (h + 1)]
        bv[0, 65 * h:65 * h + 64] = b1[512 + 64 * h:512 + 64 * (h + 1)]
        bv[0, 65 * h + 64] = 1.0
    fc_bt = np.ascontiguousarray(b2.reshape(FT, 128).T)

    import ml_dtypes
    bf = ml_dtypes.bfloat16

    return {
        "qk_w": qk_w.astype(bf),
        "qk_bc": qk_bc,
        "wv": wv.astype(bf),
        "bv_row": bv,
        "out_w": np.ascontiguousarray(
            np.asarray(inputs["out_w"], f).reshape(H, D, E)
            .transpose(1, 0, 2)).astype(bf),
        "outb_row": np.asarray(inputs["out_b"], f).reshape(1, E),
        "fc_w": W2.astype(bf),
        "fc_bt": fc_bt,
        "proj_w": np.asarray(inputs["proj_w"], f).astype(bf),
        "projb_row": np.asarray(inputs["proj_b"], f).reshape(1, E).astype(bf),
    }


_NC_CACHE = None


def _get_nc():
    global _NC_CACHE
    if _NC_CACHE is None:
        _NC_CACHE = build_bass()
    return _NC_CACHE


def run(inputs, trace=False):
    from concourse.bass_utils import run_bass_kernel_spmd

    nc = _get_nc()
    weights = host_prep(inputs)
    x = np.asarray(inputs["x"], np.float32)
    in_maps = []
    for c in range(NCORES):
        m = dict(weights)
        m["x"] = np.ascontiguousarray(x[BPC * c:BPC * (c + 1)])
        in_maps.append(m)
    res = run_bass_kernel_spmd(nc, in_maps, core_ids=list(range(NCORES)),
                               trace=trace)
    y = np.concatenate([res.results[c]["y"] for c in range(NCORES)], axis=0)
    return y, res


def kernel(**inputs):
    y, _ = run(inputs)
    return y


# revision 15
# speedup vs baseline: 1.1804x; 1.0624x over previous
"""Trainium2 Bass kernel for a GPT-style transformer block.

Reference computation (per batch element):
    h  = LN1(x);  qkv = h @ qkv_w + qkv_b
    att = causal_softmax(q @ k.T / sqrt(64));  o = att @ v
    x  = x + o @ out_w + out_b
    h  = LN2(x);  u = relu(h @ fc_w + fc_b)
    y  = x + u @ proj_w + proj_b

Shapes: x [16, 1024, 256], 4 heads x 64, MLP hidden 1024.

Strategy: pure data-parallel over batch, 2 batch elements per core on 8
cores, no collectives.  Within a core:
  - LN gamma/beta folded into the following matmul weights on the host.
  - The residual stream (xs/x2) and LN math stay fp32; matmul operands
    (h1T/qkT/vv/expT/uT + weights) are bf16 except out_proj (f32r).
  - Attention scores are computed transposed (scoresT[k, q]) so the
    causal region is per-k-tile a contiguous q-range.  The causal mask is
    ADDITIVE (-1e30) applied by DVE on the PSUM diagonal block before the
    scalar-engine exp, so P@V waits only on exps.  Softmax denominators
    come from an extra all-ones column appended to V (P@V_aug also yields
    row sums); normalization uses a sums -> reciprocal DRAM round trip
    that lands per-partition reciprocal columns broadcast over the o rows.
  - Biases ride evacuation ops (DVE adds against broadcast bias tiles,
    out_b pre-added into the residual) instead of rank-1 PE matmuls
    (except proj_b, whose residual target also feeds LN2).
  - Emission is generator-driven and interleaved so each engine's
    in-order queue rarely has a blocked op ahead of ready work:
    p1(0)+p1(1) | p2(0)+p2(1) | p2(1)+p3(0) | p3(0)+p3(1) | p3(1).
"""

import sys

sys.path.insert(0, "/opt/trn_rl_repo")

import numpy as np

import concourse.bass as bass
import concourse.bacc as bacc
import concourse.tile as tile
from concourse import mybir
from concourse.masks import make_identity, make_upper_triangular

# The activation-set chooser is first-match, which thrashes table loads
# between the exp and ln sets.  natural_log_exp_and_others contains every
# function this kernel uses (exp, ln, copy/identity, relu), so restrict
# the chooser to it (other entries stay in place so set ids keep their
# positions for walrus).
_orig_get_act_tables = bacc.get_activation_tables


def _one_set_tables(module_arch):
    tabs = _orig_get_act_tables(module_arch)
    return {name: (fns if name == "natural_log_exp_and_others" else set())
            for name, fns in tabs.items()}


bacc.get_activation_tables = _one_set_tables

F32 = mybir.dt.float32
F32R = mybir.dt.float32r
BF16 = mybir.dt.bfloat16
AF = mybir.ActivationFunctionType
ALU = mybir.AluOpType

NCORES = 8
B = 16
BPC = B // NCORES  # 2 batch elements per core
S = 1024
E = 256
H = 4
D = 64
FF = 1024
ST = S // 128  # 8 seq tiles
ET = E // 128  # 2 feature tiles
FT = FF // 128  # 8 mlp-hidden tiles
EPS = 1e-5
NEG = -1.0e30

# ragged offsets for the causal expT store: tile j holds q in [128j, S)
EOFF = [0]
for _j in range(ST):
    EOFF.append(EOFF[-1] + (S - 128 * _j))
ETOT = EOFF[ST]  # 4608


def _r(ap):
    """View an fp32 AP as float32r for full-rate PE matmuls (no-op for
    tiles already in a matmul dtype)."""
    if ap.dtype in (F32R, BF16):
        return ap
    return ap.bitcast(F32R)


def _bcast(ap_1d, parts):
    """Broadcast a 1-partition AP across `parts` partitions (step-0 AP)."""
    return bass.AP(tensor=ap_1d.tensor, offset=ap_1d.offset,
                   ap=[[0, parts]] + list(ap_1d.ap))


def _drain_until(stop_gen, *gens):
    """Round-robin stop_gen plus (gen, weight) extras; return when
    stop_gen exhausts (extras keep their state for later drains)."""
    live = [[g, w] for g, w in gens]
    while True:
        try:
            next(stop_gen)
        except StopIteration:
            return
        nxt = []
        for g, w in live:
            alive = True
            for _ in range(w):
                try:
                    next(g)
                except StopIteration:
                    alive = False
                    break
            if alive:
                nxt.append([g, w])
        live = nxt


def build_bass(reps=1):
    nc = bacc.Bacc(None, target_bir_lowering=False, debug=False)

    # ---- DRAM I/O ----
    x_in = nc.dram_tensor("x", [BPC, S, E], F32, kind="ExternalInput")
    qk_w = nc.dram_tensor("qk_w", [E, 512], BF16, kind="ExternalInput")
    qk_bc = nc.dram_tensor("qk_bc", [128, 4], F32, kind="ExternalInput")
    wv = nc.dram_tensor("wv", [E, H * (D + 1)], BF16, kind="ExternalInput")
    bv_row = nc.dram_tensor("bv_row", [1, H * (D + 1)], F32, kind="ExternalInput")
    out_w = nc.dram_tensor("out_w", [D, H, E], BF16, kind="ExternalInput")
    outb_row = nc.dram_tensor("outb_row", [1, E], F32, kind="ExternalInput")
    fc_w = nc.dram_tensor("fc_w", [E, FF], BF16, kind="ExternalInput")
    fc_bt = nc.dram_tensor("fc_bt", [128, FT], F32, kind="ExternalInput")
    proj_w = nc.dram_tensor("proj_w", [FF, E], BF16, kind="ExternalInput")
    projb_row = nc.dram_tensor("projb_row", [1, E], BF16, kind="ExternalInput")
    y_out = nc.dram_tensor("y", [BPC, S, E], F32, kind="ExternalOutput")

    VA = H * (D + 1)  # 260

    with tile.TileContext(nc) as tc:
        wp = tc.alloc_tile_pool(name="weights", bufs=1)
        sp = tc.alloc_tile_pool(name="small", bufs=2)
        bp2 = tc.alloc_tile_pool(name="big2", bufs=2)
        bp1 = tc.alloc_tile_pool(name="big1", bufs=1)
        psA = tc.alloc_tile_pool(name="psA", bufs=2, space="PSUM")
        psS = tc.alloc_tile_pool(name="psS", bufs=4, space="PSUM")
        psO = tc.alloc_tile_pool(name="psO", bufs=2, space="PSUM")

        # ---- persistent weights / constants ----
        qk_w_sb = wp.tile([128, ET, 512], BF16)
        nc.gpsimd.dma_start(out=qk_w_sb, in_=qk_w[:, :].rearrange("(t p) c -> p t c", p=128))
        qk_bc_sb = wp.tile([128, 4], F32)
        nc.gpsimd.dma_start(out=qk_bc_sb, in_=qk_bc[:, :])
        wv_sb = wp.tile([128, ET, VA], BF16)
        nc.gpsimd.dma_start(out=wv_sb, in_=wv[:, :].rearrange("(t p) c -> p t c", p=128))
        bv_sb = wp.tile([128, VA], F32)
        nc.gpsimd.dma_start(out=bv_sb, in_=_bcast(bv_row[0, :], 128))
        out_w_sb = wp.tile([64, H, E], BF16)
        nc.gpsimd.dma_start(out=out_w_sb, in_=out_w[:, :, :])
        outb_sb = wp.tile([128, E], F32)
        nc.gpsimd.dma_start(out=outb_sb, in_=_bcast(outb_row[0, :], 128))
        fc_w_sb = wp.tile([128, ET, FF], BF16)
        nc.gpsimd.dma_start(out=fc_w_sb, in_=fc_w[:, :].rearrange("(t p) c -> p t c", p=128))
        fc_bt_sb = wp.tile([128, FT], F32)
        nc.gpsimd.dma_start(out=fc_bt_sb, in_=fc_bt[:, :])
        proj_w_sb = wp.tile([128, FT, E], BF16)
        nc.gpsimd.dma_start(out=proj_w_sb, in_=proj_w[:, :].rearrange("(t p) c -> p t c", p=128))
        projb_sb = wp.tile([1, E], BF16)
        nc.gpsimd.dma_start(out=projb_sb, in_=projb_row[:, :])

        ones_row = wp.tile([1, 128], BF16)
        nc.vector.memset(ones_row, 1.0)
        ones_col = wp.tile([128, 64], BF16)
        nc.vector.memset(ones_col, 1.0)
        eps_col = wp.tile([128, 1], F32)
        nc.vector.memset(eps_col, EPS)
        ident = wp.tile([128, 128], BF16)
        make_identity(nc, ident)
        # multiplicative causal mask (bf16): 1 where q >= k, else 0
        trimask = wp.tile([128, 128], BF16)
        make_upper_triangular(nc, trimask, val=1.0, diag=True)

        def ln_stats(src, t, stats, mv):
            nc.vector.bn_stats(out=stats[:, t, :], in_=src[:, t, :])
            nc.vector.bn_aggr(out=mv[:, t, :], in_=stats[:, t, :])

        def ln_coeffs(mv, rstd, nmr):
            # rstd = exp(-0.5 * ln(var + eps)); Ln/Exp share an ACT table set
            nc.scalar.activation(rstd, mv[:, :, 1], AF.Ln, bias=eps_col)
            nc.scalar.activation(rstd, rstd, AF.Exp, scale=-0.5)
            nc.vector.tensor_mul(nmr, mv[:, :, 0], rstd)
            nc.vector.tensor_scalar_mul(nmr, nmr, -1.0)

        def ln_apply(src, t, rstd, nmr, dst):
            nc.vector.tensor_scalar(
                out=dst, in0=src[:, t, :],
                scalar1=rstd[:, t:t + 1], scalar2=nmr[:, t:t + 1],
                op0=ALU.mult, op1=ALU.add)

        def gen_p1(b, xs, qkT, vv):
            """LN1 -> h1 -> h1T; qkT; vv.  Also xs += out_b (pre-bias for
            the out_proj residual) once LN1 has consumed raw xs."""
            for t in range(ST):
                nc.sync.dma_start(out=xs[:, t, :],
                                  in_=x_in[b, 128 * t:128 * (t + 1), :])
            stats = sp.tile([128, ST, 6], F32, tag="bnstats")
            mv = sp.tile([128, ST, 2], F32, tag="bnaggr")
            rstd = sp.tile([128, ST], F32, tag="rstd")
            nmr = sp.tile([128, ST], F32, tag="nmr")
            h1 = bp2.tile([128, ST, E], BF16, tag="h1")
            h1T = bp2.tile([128, ET, S], BF16, tag="hT")
            # LN1 pipelined per 4-tile group: stats -> coeffs -> apply ->
            # transposes of the group, so the PE starts after 4 tiles
            for g in range(2):
                ts = range(4 * g, 4 * g + 4)
                for t in ts:
                    ln_stats(xs, t, stats, mv)
                yield
                gsl = slice(4 * g, 4 * g + 4)
                ln_coeffs(mv[:, gsl, :], rstd[:, gsl], nmr[:, gsl])
                for t in ts:
                    ln_apply(xs, t, rstd, nmr, h1[:, t, :])
                    # fold out_b into the residual (LN1 done with xs[t])
                    nc.gpsimd.tensor_add(xs[:, t, :], xs[:, t, :], outb_sb)
                yield
                for e in range(ET):
                    ps = psA.tile([128, 512], BF16, tag="psA")
                    for i in range(4):
                        t = 4 * g + i
                        nc.tensor.transpose(
                            ps[:, 128 * i:128 * (i + 1)],
                            h1[:, t, 128 * e:128 * (e + 1)],
                            ident)
                    nc.scalar.copy(h1T[:, e, 512 * g:512 * (g + 1)], ps)
                    yield
            # qkT: m=0: q heads 0-1, m=1: q heads 2-3, m=2: k heads 0-1,
            # m=3: k heads 2-3 (rows = 2x64 head dims)
            for m in range(4):
                for c in range(2):
                    sl = slice(512 * c, 512 * (c + 1))
                    ps = psS.tile([128, 512], F32, tag="psS")
                    nc.tensor.matmul(ps, qk_w_sb[:, 0, 128 * m:128 * (m + 1)],
                                     h1T[:, 0, sl], start=True, stop=False)
                    nc.tensor.matmul(ps, qk_w_sb[:, 1, 128 * m:128 * (m + 1)],
                                     h1T[:, 1, sl], start=False, stop=True)
                    nc.vector.tensor_scalar(
                        out=qkT[:, m, sl], in0=ps, scalar1=qk_bc_sb[:, m:m + 1],
                        scalar2=None, op0=ALU.add)
                yield
            # v seq-major with per-head all-ones column: vv[:, t, 65h+64] == 1
            # (bias + the 1.0 column ride the DVE evacuation via bv_sb)
            for t in range(ST):
                ps = psA.tile([128, 512], F32, tag="psA")
                nc.tensor.matmul(ps[:, 0:VA], h1T[:, 0, 128 * t:128 * (t + 1)],
                                 wv_sb[:, 0, :], start=True, stop=False)
                nc.tensor.matmul(ps[:, 0:VA], h1T[:, 1, 128 * t:128 * (t + 1)],
                                 wv_sb[:, 1, :], start=False, stop=True)
                nc.vector.tensor_add(vv[:, t, :], ps[:, 0:VA], bv_sb)
                if t % 2 == 1:
                    yield

        def gen_p2(b, qkT, vv, oTh):
            """Attention: scoresT (+additive causal mask on PSUM) -> exp ->
            P@V_aug -> softmax normalization via a PE rank-1 broadcast of
            the sums row and a DVE divide (no DMA round trips)."""
            for pair in range(2):
                expTs = {}
                oscs = {}
                for hh in range(2):
                    h = 2 * pair + hh
                    expTs[h] = bp2.tile([128, ETOT], BF16, tag="expT",
                                        name=f"expT{h}", bufs=4)
                    oscs[h] = sp.tile([65, 2, 512], BF16, tag="osc",
                                      name=f"osc{h}", bufs=4)

                def scores_j(j):
                    w0 = 128 * j
                    pss = {}
                    for hh in range(2):
                        h = 2 * pair + hh
                        qsl = slice(64 * hh, 64 * hh + 64)
                        for c in range(w0 // 512, 2):
                            a = max(w0, 512 * c)
                            ps = psS.tile([128, 512], F32, tag="psS",
                                          name=f"psS{h}c{c}")
                            pss[(h, c)] = ps
                            nc.tensor.matmul(
                                ps[:, a - 512 * c:512],
                                qkT[qsl, 2 + pair, w0:w0 + 128],
                                qkT[qsl, pair, a:512 * (c + 1)],
                                start=True, stop=True)
                    for c in range(w0 // 512, 2):
                        a = max(w0, 512 * c)
                        for hh in range(2):
                            h = 2 * pair + hh
                            nc.scalar.activation(
                                expTs[h][:, EOFF[j] + a - w0:
                                         EOFF[j] + 512 * (c + 1) - w0],
                                pss[(h, c)][:, a - 512 * c:512],
                                AF.Exp, scale=0.125)
                            if a == w0:
                                # causal mask on the diagonal block; odd
                                # heads ride the otherwise-idle gpsimd
                                eng = nc.vector if hh == 0 else nc.gpsimd
                                eng.tensor_mul(
                                    expTs[h][:, EOFF[j]:EOFF[j] + 128],
                                    expTs[h][:, EOFF[j]:EOFF[j] + 128],
                                    trimask)

                def attv(hh, c):
                    h = 2 * pair + hh
                    expT = expTs[h]
                    po = psO.tile([128, 512], F32, tag="psO")
                    jmax = 4 * (c + 1)
                    for j in range(jmax):
                        a = max(128 * j, 512 * c)
                        nc.tensor.matmul(
                            po[0:65, a - 512 * c:512],
                            vv[:, j, 65 * h:65 * (h + 1)],
                            expT[:, EOFF[j] + a - 128 * j:
                                 EOFF[j] + 512 * (c + 1) - 128 * j],
                            start=(j == 0), stop=(j == jmax - 1))
                    # unnormalized o rows + denominator row -> SBUF scratch
                    nc.vector.tensor_copy(oscs[h][:, c, :], po[0:65, :])
                    # broadcast the sums row over 64 partitions on the PE
                    # (rank-1 ones_col x sums_row), then normalize with a
                    # DVE divide -- no DMA round trip
                    sb = psO.tile([128, 512], F32, tag="psO", name="sb")
                    nc.tensor.matmul(sb[0:64, :], ones_col[64:65, :],
                                     oscs[h][64:65, c, :],
                                     start=True, stop=True)
                    rb = sp.tile([64, 512], F32, tag="rb", bufs=4)
                    nc.vector.reciprocal_approx_fast(out=rb, in_=sb[0:64, :])
                    eng = nc.vector if hh == 0 else nc.gpsimd
                    eng.tensor_mul(
                        oTh[h][:, 512 * c:512 * (c + 1)],
                        oscs[h][0:64, c, :], rb)

                for j in range(4):
                    scores_j(j)
                    yield
                for hh in range(2):
                    attv(hh, 0)
                    yield
                for j in range(4, ST):
                    scores_j(j)
                    yield
                for hh in range(2):
                    attv(hh, 1)
                    yield

        def gen_p3a(b, xs, oTh, x2, st3):
            """out_proj + residual -> x2; LN2 -> h2 -> h2T (stored into
            st3['h2T'] for gen_p3b)."""
            stats = sp.tile([128, ST, 6], F32, tag="bnstats")
            mv = sp.tile([128, ST, 2], F32, tag="bnaggr")
            for t in range(ST):
                ps = psA.tile([128, 512], F32, tag="psA")
                for h in range(H):
                    nc.tensor.matmul(ps[:, 0:E],
                                     oTh[h][:, 128 * t:128 * (t + 1)],
                                     out_w_sb[:, h, :],
                                     start=(h == 0), stop=(h == H - 1))
                # xs already carries +out_b
                nc.vector.tensor_add(x2[:, t, :], ps[:, 0:E], xs[:, t, :])
                ln_stats(x2, t, stats, mv)
                if t % 2 == 1:
                    yield
            rstd = sp.tile([128, ST], F32, tag="rstd")
            nmr = sp.tile([128, ST], F32, tag="nmr")
            h2 = bp2.tile([128, ST, E], BF16, tag="h1")
            h2T = bp2.tile([128, ET, S], BF16, tag="hT")
            st3["h2T"] = h2T
            for g in range(2):
                gsl = slice(4 * g, 4 * g + 4)
                ln_coeffs(mv[:, gsl, :], rstd[:, gsl], nmr[:, gsl])
                for t in range(4 * g, 4 * g + 4):
                    ln_apply(x2, t, rstd, nmr, h2[:, t, :])
                yield
                for e in range(ET):
                    ps = psA.tile([128, 512], BF16, tag="psA")
                    for i in range(4):
                        t = 4 * g + i
                        nc.tensor.transpose(
                            ps[:, 128 * i:128 * (i + 1)],
                            h2[:, t, 128 * e:128 * (e + 1)],
                            ident)
                    nc.scalar.copy(h2T[:, e, 512 * g:512 * (g + 1)], ps)
                    yield

        def gen_p3b(b, xs, x2, st3):
            """MLP fc + relu -> uT; proj + residual -> y (reuses xs)."""
            h2T = st3["h2T"]
            # ---- MLP fc + relu (uT feature-major) ----
            uT = bp1.tile([128, FT, S], BF16, tag="uT")
            for m in range(FT):
                for c in range(2):
                    sl = slice(512 * c, 512 * (c + 1))
                    ps = psA.tile([128, 512], F32, tag="psA")
                    nc.tensor.matmul(ps, fc_w_sb[:, 0, 128 * m:128 * (m + 1)],
                                     h2T[:, 0, sl], start=True, stop=False)
                    nc.tensor.matmul(ps, fc_w_sb[:, 1, 128 * m:128 * (m + 1)],
                                     h2T[:, 1, sl], start=False, stop=True)
                    if c == 0:
                        nc.scalar.activation(uT[:, m, sl], ps, AF.Relu,
                                             bias=fc_bt_sb[:, m:m + 1])
                    else:
                        nc.vector.tensor_scalar(
                            out=uT[:, m, sl], in0=ps,
                            scalar1=fc_bt_sb[:, m:m + 1], scalar2=0.0,
                            op0=ALU.add, op1=ALU.max)
                yield
            # ---- proj + residual -> y (reuses the xs tile in place) ----
            for t in range(ST):
                ps = psA.tile([128, 512], F32, tag="psA")
                for m in range(FT):
                    nc.tensor.matmul(ps[:, 0:E], uT[:, m, 128 * t:128 * (t + 1)],
                                     proj_w_sb[:, m, :],
                                     start=(m == 0), stop=False)
                nc.tensor.matmul(ps[:, 0:E], ones_row,
                                 projb_sb[0:1, :], start=False, stop=True)
                nc.vector.tensor_add(xs[:, t, :], ps[:, 0:E], x2[:, t, :])
                nc.sync.dma_start(out=y_out[b, 128 * t:128 * (t + 1), :],
                                  in_=xs[:, t, :])
                yield

        for _rep in range(reps):
            xs_t, qkT_t, vv_t, oTh_t, x2_t = [], [], [], [], []
            for b in range(BPC):
                xs_t.append(bp2.tile([128, ST, E], F32, tag="xs",
                                     name=f"xs{b}"))
                qkT_t.append(bp1.tile([128, 4, S], BF16, tag="qkT",
                                      name=f"qkT{b}", bufs=2))
                vv_t.append(bp1.tile([128, ST, VA], BF16, tag="vv",
                                     name=f"vv{b}", bufs=2))
                oTh_t.append([bp1.tile([64, S], BF16, tag="oTh",
                                       name=f"oTh{b}_{h}", bufs=8)
                              for h in range(H)])
                x2_t.append(bp1.tile([128, ST, E], F32, tag="x2",
                                     name=f"x2{b}", bufs=2))
            st3 = [{}, {}]

            g10 = gen_p1(0, xs_t[0], qkT_t[0], vv_t[0])
            g11 = gen_p1(1, xs_t[1], qkT_t[1], vv_t[1])
            g20 = gen_p2(0, qkT_t[0], vv_t[0], oTh_t[0])
            g21 = gen_p2(1, qkT_t[1], vv_t[1], oTh_t[1])
            g30a = gen_p3a(0, xs_t[0], oTh_t[0], x2_t[0], st3[0])
            g30b = gen_p3b(0, xs_t[0], x2_t[0], st3[0])
            g31a = gen_p3a(1, xs_t[1], oTh_t[1], x2_t[1], st3[1])
            g31b = gen_p3b(1, xs_t[1], x2_t[1], st3[1])

            def _chain(*gens):
                for g in gens:
                    yield from g

            g30 = _chain(g30a, g30b)
            g31 = _chain(g31a, g31b)

            # Chained emission schedule: each drain stops when its leader
            # exhausts; partners carry over with their state.  Attention of
            # one batch is co-scheduled with the PE-dense p1/p3 work of the
            # OTHER batch so the PE stays continuously busy (p-state) while
            # the scalar engine chews exps.
            _drain_until(g10, (g11, 1))
            _drain_until(g20, (g11, 1))
            _drain_until(g21, (g30, 1))
            _drain_until(g30)
            _drain_until(g31)

        for p in (psO, psS, psA, bp1, bp2, sp, wp):
            p.release()

    nc.compile()
    return nc


def host_prep(inputs):
    """Fold LN params into weights; build the DRAM-side weight layouts."""
    f = np.float32
    qkv_w = np.asarray(inputs["qkv_w"], f)
    qkv_b = np.asarray(inputs["qkv_b"], f)
    ln1_g = np.asarray(inputs["ln1_g"], f)
    ln1_b = np.asarray(inputs["ln1_b"], f)
    ln2_g = np.asarray(inputs["ln2_g"], f)
    ln2_b = np.asarray(inputs["ln2_b"], f)
    fc_w = np.asarray(inputs["fc_w"], f)
    fc_b = np.asarray(inputs["fc_b"], f)

    W1 = qkv_w * ln1_g[:, None]
    b1 = qkv_b + ln1_b @ qkv_w
    W2 = fc_w * ln2_g[:, None]
    b2 = fc_b + ln2_b @ fc_w

    qk_w = np.ascontiguousarray(W1[:, :512])
    qk_bc = np.ascontiguousarray(b1[:512].reshape(4, 128).T)
    wv = np.zeros((E, H * (D + 1)), f)
    bv = np.zeros((1, H * (D + 1)), f)
    for h in range(H):
        wv[:, 65 * h:65 * h + 64] = W1[:, 512 + 64 * h:512 + 64 * (h + 1)]
        bv[0, 65 * h:65 * h + 64] = b1[512 + 64 * h:512 + 64 * (h + 1)]
        bv[0, 65 * h + 64] = 1.0
    fc_bt = np.ascontiguousarray(b2.reshape(FT, 128).T)

    import ml_dtypes
    bf = ml_dtypes.bfloat16

    return {
        "qk_w": qk_w.astype(bf),
        "qk_bc": qk_bc,
        "wv": wv.astype(bf),
        "bv_row": bv,
        "out_w": np.ascontiguousarray(
            np.asarray(inputs["out_w"], f).reshape(H, D, E)
            .transpose(1, 0, 2)).astype(bf),
        "outb_row": np.asarray(inputs["out_b"], f).reshape(1, E),
        "fc_w": W2.astype(bf),
        "fc_bt": fc_bt,
        "proj_w": np.asarray(inputs["proj_w"], f).astype(bf),
        "projb_row": np.asarray(inputs["proj_b"], f).reshape(1, E).astype(bf),
    }


_NC_CACHE = None


def _get_nc():
    global _NC_CACHE
    if _NC_CACHE is None:
        _NC_CACHE = build_bass()
    return _NC_CACHE


def run(inputs, trace=False):
    from concourse.bass_utils import run_bass_kernel_spmd

    nc = _get_nc()
    weights = host_prep(inputs)
    x = np.asarray(inputs["x"], np.float32)
    in_maps = []
    for c in range(NCORES):
        m = dict(weights)
        m["x"] = np.ascontiguousarray(x[BPC * c:BPC * (c + 1)])
        in_maps.append(m)
    res = run_bass_kernel_spmd(nc, in_maps, core_ids=list(range(NCORES)),
                               trace=trace)
    y = np.concatenate([res.results[c]["y"] for c in range(NCORES)], axis=0)
    return y, res


def kernel(**inputs):
    y, _ = run(inputs)
    return y
